# revision 43
# baseline (speedup 1.0000x reference)
"""Trainium2 Bass kernel for nn_CausalSelfAttention_7559142441541.

Sliding-window (MEMORY=64) causal self-attention that also returns the full
causal attention probabilities `att` [B,H,T,T] and their batch-std
`att_std` [H,T,T] (ddof=1).

Sharding: 2 heads per NeuronCore across 8 cores. `att`/`att_std` are
head-local; y = window_attn @ W_proj reduces over heads, so each core
produces a partial y (its 128 rows of W_proj) and the host sums the 8
partials.

Per-core device algorithm (heads h0=2c, h1=2c+1), one fused streaming loop
over eight 128-token column blocks so qkv production, causal-att/std
write-out (DMA-bound) and band-attention+proj (PE-bound) all overlap:
  qkv:   qT/kT [128, B*T] (2 heads stacked on partitions) and v tiles
         [t%%128, tblk, h, 65] (65th column = 1.0 -> the band softmax
         denominator falls out of the same A@V matmul) computed from
         xT [C, B*T] with batch-interleaved column blocks, so attention on
         block j starts as soon as blocks <= j are produced.
  att:   causal scores strip via PE; the diagonal causal mask and the band
         mask are ADDED in PSUM by accumulating identity-matmuls (tensor
         engine, not DVE); one exp on ACT per strip with accum_out giving
         the row-sum for free (unshifted softmax - scores are O(1) so exp
         cannot overflow); batched reciprocal; att = E * rinv.
         Upper-triangle blocks are never written (NEFF outputs are donated
         zero buffers, so unwritten DRAM reads back as zeros).
  std:   one-pass batch variance V = (sum a^2 - (sum a)^2/4), clamped at 0,
         std = sqrt(V/3) on ACT; adds split across DVE and GpSimd.
  y:     band scores computed TRANSPOSED [j, i] so P@v contracts without
         on-chip transposes; per-row 1/bandsum broadcast to 64 partitions
         via a K=1 ones-matmul; per-head K=64 proj matmuls accumulate both
         heads into PSUM.

Matmuls run as float32r (full PE rate at moving-dim >= 256; ~19-bit
mantissa, giving ~2-4e-4 relative error vs the fp32 reference). All
matmul operands are produced as f32r (DMA-typed inputs or casting copies)
to satisfy the BIR verifier's rounding rule.
"""

import numpy as np

import bass_rust
import concourse.bass as bass
import concourse.tile as tile
from concourse import mybir
from concourse.bass import ts
from concourse.bass_utils import run_bass_kernel_spmd
from concourse.masks import make_identity

B, T, C = 4, 1024, 1024
NH = 16
HD = 64
MEM = 64
SCALE = 1.0 / float(np.sqrt(HD))
F32 = mybir.dt.float32
F32R = mybir.dt.float32r
EXP = mybir.ActivationFunctionType.Exp
SQRT = mybir.ActivationFunctionType.Sqrt
SQUARE = mybir.ActivationFunctionType.Square
GE = mybir.AluOpType.is_ge
AXX = mybir.AxisListType.X

# The walrus build in this environment rejects instructions carrying more
# than one sync-wait command (setupSyncWait in CoreV3GenImpl). Tile freely
# emits multi-wait instructions, so after scheduling we rewrite the BIR:
# each excess wait moves to a fresh NoOp on the same engine placed directly
# before the instruction. Engines execute their stream in order, so the
# accumulated waits gate the instruction exactly as before.
_MAX_WAITS = 1


def _split_waits(nc, max_waits=_MAX_WAITS):
    n_new = 0
    for fn in nc.m.functions:
        for bb in fn.blocks:
            out = []
            changed = False
            for inst in bb.instructions:
                si = inst.sync_info
                if si is not None:
                    waits = list(si.on_wait)
                    if len(waits) > max_waits:
                        changed = True
                        keep = len(waits) - max_waits
                        for i, w in enumerate(waits[:keep]):
                            nop = mybir.InstNoOp(
                                name=f"{inst.name}__w{i}", ins=[], outs=[]
                            )
                            nop.engine = inst.engine
                            nop.sync_info = bass_rust.SyncInfo(
                                on_wait=[w], on_update=[]
                            )
                            nc.register_instruction(nop)
                            out.append(nop)
                            n_new += 1
                        si.on_wait = waits[keep:]
                out.append(inst)
            if changed:
                bb.instructions = out
    return n_new


def _r(ap):
    return ap.bitcast(F32R)


def build_nc(phases=3):
    nc = bass.Bass("TRN2", target_bir_lowering=False, debug=False, num_devices=8)
    xT_d = nc.dram_tensor("xT", [C, B * T], F32R, kind="ExternalInput").ap()
    wa_d = nc.dram_tensor("wa", [C, 384], F32R, kind="ExternalInput").ap()
    wp_d = nc.dram_tensor("wp", [128, C], F32R, kind="ExternalInput").ap()
    att_d = nc.dram_tensor("att_c", [B, 2, T, T], F32, kind="ExternalOutput").ap()
    std_d = nc.dram_tensor("std_c", [2, T, T], F32, kind="ExternalOutput").ap()
    yp_d = nc.dram_tensor("yp_c", [B, T, C], F32, kind="ExternalOutput").ap()
    NEG = -700.0  # additive mask: exp((s + NEG) * SCALE) == 0 in fp32

    # xT viewed as [ki, ko, b, jblk, 128]: phase 1 streams column-block jblk
    # for ALL batches at once, so attention on block jblk can start as soon
    # as blocks <= jblk are done.
    xT_r = xT_d.rearrange(
        "(ko ki) (bb jb t) -> ki ko bb jb t", ki=128, bb=B, t=128
    )

    with tile.TileContext(nc) as tc:
        with (
            tc.tile_pool(name="singles", bufs=1) as singles,
            tc.tile_pool(name="p1", bufs=3) as p1,
            tc.tile_pool(name="p2", bufs=2) as p2,
            tc.tile_pool(name="p2e", bufs=4) as p2e,
            tc.tile_pool(name="p3", bufs=2) as p3,
            tc.tile_pool(name="ps", bufs=2, space="PSUM") as ps,
        ):
            wa_sb = singles.tile([128, 8, 384], F32R)
            nc.sync.dma_start(
                wa_sb[:], wa_d.rearrange("(ko ki) f -> ki ko f", ki=128)
            )
            wp_sb = singles.tile([128, C], F32R)
            nc.sync.dma_start(wp_sb[:], wp_d)
            # W_proj rows 64:128 replicated at partitions 0:64 so the h1 proj
            # matmul can run with both operands at partition base 0.
            wp2_sb = singles.tile([64, C], F32R)
            nc.sync.dma_start(wp2_sb[:], wp_d[64:128, :])
            q_sb = singles.tile([128, B * T], F32R)
            k_sb = singles.tile([128, B * T], F32R)
            v_sb = singles.tile([128, 32, 2, 65], F32R)
            nc.vector.memset(v_sb[:, :, :, 64:65].bitcast(F32), 1.0)
            ones_sb = singles.tile([128, 64], F32R)
            nc.vector.memset(ones_sb[:].bitcast(F32), 1.0)
            ident = singles.tile([128, 128], F32)
            make_identity(nc, ident[:])
            scratch = singles.tile([128, 512], F32)
            ident_r = singles.tile([128, 128], F32R)
            make_identity(nc, scratch[:, 0:128])
            nc.vector.tensor_copy(ident_r[:], scratch[:, 0:128])
            # Masks are ADDED to the psum score tiles via accumulating
            # identity-matmuls on the tensor engine.
            # maskw: [128, 512] with the causal diagonal mask (0 where j<=i,
            # NEG above) in the LAST 128 columns, zeros elsewhere, so the
            # mask-matmul covers the full range of the final score matmul.
            maskw = singles.tile([128, 512], F32R)
            mw = scratch[:]
            nc.vector.memset(mw, 0.0)
            nc.gpsimd.affine_select(
                mw[:, 384:512], mw[:, 384:512], [[-1, 128]], GE, NEG,
                base=0, channel_multiplier=1,
            )
            nc.vector.tensor_copy(maskw[:], mw)
            # additive band mask for the [j, i] band-score tile layout
            # segments: [0:128]=m1 (keep p - fi - 64 >= 0),
            # [128:384]=m0 and [384:512]=p1 (keep 0 <= i-j <= 64)
            band_madd = singles.tile([128, 512], F32R)
            bm = scratch[:]
            nc.vector.memset(bm, 0.0)
            nc.gpsimd.affine_select(
                bm[:, 0:128], bm[:, 0:128], [[-1, 128]], GE,
                NEG, base=-64, channel_multiplier=1,
            )
            nc.gpsimd.affine_select(
                bm[:, 128:384], bm[:, 128:384], [[1, 256]], GE,
                NEG, base=0, channel_multiplier=-1,
            )
            nc.gpsimd.affine_select(
                bm[:, 128:384], bm[:, 128:384], [[-1, 256]], GE,
                NEG, base=64, channel_multiplier=1,
            )
            nc.gpsimd.affine_select(
                bm[:, 384:512], bm[:, 384:512], [[1, 128]], GE,
                NEG, base=0, channel_multiplier=-1,
            )
            nc.gpsimd.affine_select(
                bm[:, 384:512], bm[:, 384:512], [[-1, 128]], GE,
                NEG, base=64, channel_multiplier=1,
            )
            nc.vector.tensor_copy(band_madd[:], bm)

            def phase1_block(jb):
                """qkv for column block jb (128 tokens of each batch)."""
                xt = p1.tile([128, 4, 4, 128], F32R, tag="xt", bufs=2)
                xt2 = p1.tile([128, 4, 4, 128], F32R, tag="xt2", bufs=2)
                for ko in range(4):
                    nc.sync.dma_start(
                        xt[:, ko, :, :], xT_r[:, ko, :, jb, :]
                    )
                    nc.sync.dma_start(
                        xt2[:, ko, :, :], xT_r[:, ko + 4, :, jb, :]
                    )
                qk = ps.tile([128, 1024], F32, tag="qk", bufs=1)
                def xsl(ko):
                    return xt[:, ko, :, :] if ko < 4 else xt2[:, ko - 4, :, :]
                for ko in range(8):
                    nc.tensor.matmul(
                        qk[:, 0:512], (wa_sb[:, ko, 0:128]), (xsl(ko)),
                        start=(ko == 0), stop=(ko == 7),
                    )
                for ko in range(8):
                    nc.tensor.matmul(
                        qk[:, 512:1024], (wa_sb[:, ko, 128:256]), (xsl(ko)),
                        start=(ko == 0), stop=(ko == 7),
                    )
                psv = ps.tile([128, 512], F32, tag="v", bufs=1)
                for ko in range(8):
                    nc.tensor.matmul(
                        psv[:], (wa_sb[:, ko, 256:384]), (xsl(ko)),
                        start=(ko == 0), stop=(ko == 7),
                    )
                # q/k columns for batch b live at b*T + jb*128; one strided
                # copy covers all four batches
                qv = q_sb[:].rearrange("p (bb jb t) -> p bb jb t", bb=B, t=128)
                kv = k_sb[:].rearrange("p (bb jb t) -> p bb jb t", bb=B, t=128)
                nc.vector.tensor_copy(
                    qv[:, :, jb, :], qk[:, 0:512].rearrange(
                        "p (bb t) -> p bb t", bb=B)
                )
                nc.scalar.copy(
                    kv[:, :, jb, :], qk[:, 512:1024].rearrange(
                        "p (bb t) -> p bb t", bb=B)
                )
                vT = p1.tile([128, 512], F32, tag="vT", bufs=2)
                nc.scalar.copy(vT[:], psv[:])
                for b in range(B):
                    pst = ps.tile([128, 128], F32, tag="v", bufs=1)
                    nc.tensor.transpose(pst[:], vT[:, ts(b, 128)], ident[:])
                    tb = b * 8 + jb
                    nc.vector.tensor_copy(v_sb[:, tb, 0, 0:64], pst[:, 0:64])
                    nc.vector.tensor_copy(
                        v_sb[:, tb, 1, 0:64], pst[:, 64:128]
                    )

            def phase2_block(iblk):
                i0 = iblk * 128
                jw = i0 + 128
                w_last = jw - (jw - 1) // 512 * 512
                jt_last = jw - w_last
                for h in range(2):
                    hs = slice(h * 64, (h + 1) * 64)
                    att4 = p2.tile([128, 4, 1024], F32, tag="att4")
                    Es = []
                    rs = p2e.tile([128, 4], F32, tag="rs")
                    for b in range(B):
                        g0 = b * T
                        q_lhsT = q_sb[hs, g0 + i0 : g0 + i0 + 128]
                        E = p2e.tile([128, 1024], F32, tag="E")
                        rs2 = p2e.tile([128, 2], F32, tag="rs2")
                        nchunk = (jw + 511) // 512
                        for ci, jt in enumerate(range(0, jw, 512)):
                            w = min(512, jw - jt)
                            psc = ps.tile([128, 512], F32, tag="sc")
                            nc.tensor.matmul(
                                psc[:, 0:w],
                                (q_lhsT),
                                (k_sb[hs, g0 + jt : g0 + jt + w]),
                                start=True, stop=(jt != jt_last),
                            )
                            if jt == jt_last:
                                nc.tensor.matmul(
                                    psc[:, 0:w], (ident_r[:]),
                                    (maskw[:, 512 - w_last : 512]),
                                    start=False, stop=True,
                                )
                            acc = (rs[:, b : b + 1] if nchunk == 1
                                   else rs2[:, ci : ci + 1])
                            nc.scalar.activation(
                                E[:, jt : jt + w], psc[:, 0:w], EXP,
                                scale=SCALE, accum_out=acc,
                            )
                        if nchunk > 1:
                            nc.vector.tensor_add(
                                rs[:, b : b + 1], rs2[:, 0:1], rs2[:, 1:2]
                            )
                        Es.append(E)
                    rinv = p2e.tile([128, 4], F32, tag="rinv")
                    nc.vector.reciprocal(rinv[:], rs[:])
                    for b in range(B):
                        eng = nc.gpsimd if b == 3 else nc.vector
                        eng.tensor_scalar_mul(
                            att4[:, b, :jw], Es[b][:, :jw],
                            rinv[:, b : b + 1],
                        )
                    deng = nc.sync if (iblk + h) % 2 == 0 else nc.scalar
                    deng.dma_start(
                        att_d[:, h, i0 : i0 + 128, 0:jw].rearrange(
                            "b p j -> p b j"
                        ),
                        att4[:, :, :jw],
                    )
                    # batch std, one-pass: V = S2 - S1*S1/4, std = sqrt(V/3)
                    a = att4[:, :, :jw]
                    sq = p2.tile([128, 4, 1024], F32, tag="sq", bufs=1)
                    nc.scalar.activation(sq[:, 0:4, :jw], a[:, 0:4], SQUARE)
                    tA = p2.tile([128, 1024], F32, tag="tA", bufs=1)
                    tB = p2.tile([128, 1024], F32, tag="tB", bufs=1)
                    tS = p2.tile([128, 1024], F32, tag="tS", bufs=1)
                    nc.gpsimd.tensor_add(tA[:, :jw], a[:, 0], a[:, 1])
                    nc.vector.tensor_add(tB[:, :jw], a[:, 2], a[:, 3])
                    nc.vector.tensor_add(tA[:, :jw], tA[:, :jw], tB[:, :jw])
                    nc.gpsimd.tensor_add(
                        tB[:, :jw], sq[:, 0, :jw], sq[:, 1, :jw]
                    )
                    nc.vector.tensor_add(
                        tS[:, :jw], sq[:, 2, :jw], sq[:, 3, :jw]
                    )
                    nc.vector.tensor_add(tS[:, :jw], tS[:, :jw], tB[:, :jw])
                    nc.vector.tensor_mul(tB[:, :jw], tA[:, :jw], tA[:, :jw])
                    nc.vector.scalar_tensor_tensor(
                        tA[:, :jw], tB[:, :jw], -0.25, tS[:, :jw],
                        op0=mybir.AluOpType.mult,
                        op1=mybir.AluOpType.add,
                    )
                    nc.vector.tensor_scalar_max(tA[:, :jw], tA[:, :jw], 0.0)
                    nc.scalar.activation(
                        tB[:, :jw], tA[:, :jw], SQRT, scale=1.0 / 3.0
                    )
                    deng2 = nc.scalar if (iblk + h) % 2 == 0 else nc.sync
                    deng2.dma_start(
                        std_d[h, i0 : i0 + 128, 0:jw], tB[:, :jw]
                    )

            def phase3_block(b, ipair):
                g0 = b * T
                i0 = ipair * 256
                yb = {}
                for h in range(2):
                    hs = slice(h * 64, (h + 1) * 64)
                    psb = ps.tile([128, 512], F32, tag="band", bufs=1)
                    # segments: [0:128]=m1, [128:384]=m0, [384:512]=p1
                    if ipair > 0:
                        nc.tensor.matmul(
                            psb[:, 0:128],
                            (k_sb[hs, g0 + i0 - 128 : g0 + i0]),
                            (q_sb[hs, g0 + i0 : g0 + i0 + 128]),
                            start=True, stop=False,
                        )
                        nc.tensor.matmul(
                            psb[:, 0:128], (ident_r[:]),
                            (band_madd[:, 0:128]),
                            start=False, stop=True,
                        )
                    nc.tensor.matmul(
                        psb[:, 128:384],
                        (k_sb[hs, g0 + i0 : g0 + i0 + 128]),
                        (q_sb[hs, g0 + i0 : g0 + i0 + 256]),
                        start=True, stop=False,
                    )
                    nc.tensor.matmul(
                        psb[:, 128:384], (ident_r[:]),
                        (band_madd[:, 128:384]),
                        start=False, stop=True,
                    )
                    nc.tensor.matmul(
                        psb[:, 384:512],
                        (k_sb[hs, g0 + i0 + 128 : g0 + i0 + 256]),
                        (q_sb[hs, g0 + i0 + 128 : g0 + i0 + 256]),
                        start=True, stop=False,
                    )
                    nc.tensor.matmul(
                        psb[:, 384:512], (ident_r[:]),
                        (band_madd[:, 384:512]),
                        start=False, stop=True,
                    )
                    lo = 0 if ipair > 0 else 128
                    ET = p3.tile([128, 512], F32R, tag="ET")
                    nc.scalar.activation(
                        ET[:, lo:512], psb[:, lo:512], EXP, scale=SCALE
                    )
                    # AV (+ bandsum via the ones column), [65, 256]
                    jt0 = b * 8 + ipair * 2
                    pav = ps.tile([128, 256], F32, tag="av", bufs=1)
                    nc.tensor.matmul(
                        pav[0:65, 0:256], (v_sb[:, jt0, h, :]),
                        (ET[:, 128:384]), start=True, stop=False,
                    )
                    if ipair > 0:
                        nc.tensor.matmul(
                            pav[0:65, 0:128], (v_sb[:, jt0 - 1, h, :]),
                            (ET[:, 0:128]), start=False, stop=False,
                        )
                    nc.tensor.matmul(
                        pav[0:65, 128:256], (v_sb[:, jt0 + 1, h, :]),
                        (ET[:, 384:512]), start=False, stop=True,
                    )
                    rb = p3.tile([128, 256], F32R, tag="rb")
                    with nc.allow_low_precision(
                        reason="f32r rounding for PE broadcast"
                    ):
                        nc.vector.reciprocal(rb[64:65, :], pav[64:65, :])
                    # broadcast rinv row to 64 partitions via a K=1 matmul:
                    # ones[1,64].T @ rinv[1,256] -> [64,256]
                    bc_ps = ps.tile([128, 256], F32, tag="band", bufs=1)
                    nc.tensor.matmul(
                        bc_ps[0:64, :], (ones_sb[64:65, :]),
                        (rb[64:65, :]), start=True, stop=True,
                        tile_position=(64, 0),
                    )
                    ybh = p3.tile([64, 256], F32R, tag="ybh")
                    nc.vector.tensor_copy(ybh[:], pav[0:64, :])
                    nc.vector.tensor_mul(ybh[:], ybh[:], bc_ps[0:64, :])
                    yb[h] = ybh
                for ic in range(2):
                    iblk = ipair * 2 + ic
                    yo = p3.tile([128, 1024], F32, tag="yo")
                    for ch in range(2):
                        psp = ps.tile([128, 512], F32, tag="proj", bufs=1)
                        nc.tensor.matmul(
                            psp[:],
                            (yb[0][:, ts(ic, 128)]),
                            (wp_sb[0:64, ts(ch, 512)]),
                            start=True, stop=False,
                        )
                        nc.tensor.matmul(
                            psp[:],
                            (yb[1][:, ts(ic, 128)]),
                            (wp2_sb[:, ts(ch, 512)]),
                            start=False, stop=True,
                        )
                        nc.scalar.copy(yo[:, ts(ch, 512)], psp[:])
                    nc.sync.dma_start(
                        yp_d[b, iblk * 128 : (iblk + 1) * 128, :], yo[:]
                    )

            for jb in range(8):
                phase1_block(jb)
                if phases >= 2:
                    phase2_block(jb)
                if phases >= 3 and jb % 2 == 1:
                    for b in range(B):
                        phase3_block(b, jb // 2)
    _split_waits(nc)
    return nc


def _shard_inputs(x, W_attn, W_proj):
    x = np.asarray(x, dtype=np.float32)
    W_attn = np.asarray(W_attn, dtype=np.float32)
    W_proj = np.asarray(W_proj, dtype=np.float32)
    xT = np.ascontiguousarray(x.reshape(B * T, C).T)
    in_maps = []
    for c in range(8):
        h0, h1 = 2 * c, 2 * c + 1
        wa = np.ascontiguousarray(
            np.concatenate(
                [
                    W_attn[:, h0 * 64 : (h0 + 1) * 64],
                    W_attn[:, h1 * 64 : (h1 + 1) * 64],
                    W_attn[:, C + h0 * 64 : C + (h0 + 1) * 64],
                    W_attn[:, C + h1 * 64 : C + (h1 + 1) * 64],
                    W_attn[:, 2 * C + h0 * 64 : 2 * C + (h0 + 1) * 64],
                    W_attn[:, 2 * C + h1 * 64 : 2 * C + (h1 + 1) * 64],
                ],
                axis=1,
            )
        )
        wp = np.ascontiguousarray(W_proj[c * 128 : (c + 1) * 128, :])
        in_maps.append({"xT": xT, "wa": wa, "wp": wp})
    return in_maps


def kernel(x, W_attn, W_proj, _trace=False):
    nc = build_nc()
    in_maps = _shard_inputs(x, W_attn, W_proj)
    res = None
    for attempt in range(4):
        try:
            res = run_bass_kernel_spmd(
                nc, in_maps, core_ids=list(range(8)), trace=_trace
            )
            break
        except Exception:
            if attempt == 3:
                raise
            import time as _time
            _time.sleep(3.0)
    att = np.concatenate(
        [r["att_c"] for r in res.results], axis=1
    )  # [B, 16, T, T]
    att_std = np.concatenate([r["std_c"] for r in res.results], axis=0)
    y = res.results[0]["yp_c"].copy()
    for r in res.results[1:]:
        y += r["yp_c"]
    kernel.last_results = res
    return (y, att, att_std)


# revision 49
# speedup vs baseline: 1.0241x; 1.0241x over previous
"""Trainium2 Bass kernel for nn_CausalSelfAttention_7559142441541.

Sliding-window (MEMORY=64) causal self-attention that also returns the full
causal attention probabilities `att` [B,H,T,T] and their batch-std
`att_std` [H,T,T] (ddof=1).

Sharding: 2 heads per NeuronCore across 8 cores. `att`/`att_std` are
head-local; y = window_attn @ W_proj reduces over heads, so each core
produces a partial y (its 128 rows of W_proj) and the host sums the 8
partials.

Per-core device algorithm (heads h0=2c, h1=2c+1), one fused streaming loop
over eight 128-token column blocks so qkv production, causal-att/std
write-out (DMA-bound) and band-attention+proj (PE-bound) all overlap:
  qkv:   qT/kT [128, B*T] (2 heads stacked on partitions) and v tiles
         [t%%128, tblk, h, 65] (65th column = 1.0 -> the band softmax
         denominator falls out of the same A@V matmul) computed from
         xT [C, B*T] with batch-interleaved column blocks, so attention on
         block j starts as soon as blocks <= j are produced.
  att:   causal scores strip via PE; the diagonal causal mask and the band
         mask are ADDED in PSUM by accumulating identity-matmuls (tensor
         engine, not DVE); one exp on ACT per strip with accum_out giving
         the row-sum for free (unshifted softmax - scores are O(1) so exp
         cannot overflow); batched reciprocal; att = E * rinv.
         Upper-triangle blocks are never written (NEFF outputs are donated
         zero buffers, so unwritten DRAM reads back as zeros).
  std:   one-pass batch variance V = (sum a^2 - (sum a)^2/4), clamped at 0,
         std = sqrt(V/3) on ACT; adds split across DVE and GpSimd.
  y:     band scores computed TRANSPOSED [j, i] so P@v contracts without
         on-chip transposes; per-row 1/bandsum broadcast to 64 partitions
         via a K=1 ones-matmul; per-head K=64 proj matmuls accumulate both
         heads into PSUM.

Matmuls run as float32r (full PE rate at moving-dim >= 256; ~19-bit
mantissa, giving ~2-4e-4 relative error vs the fp32 reference). All
matmul operands are produced as f32r (DMA-typed inputs or casting copies)
to satisfy the BIR verifier's rounding rule.
"""

import numpy as np

import bass_rust
import concourse.bass as bass
import concourse.tile as tile
from concourse import mybir
from concourse.bass import ts
from concourse.bass_utils import run_bass_kernel_spmd
from concourse.masks import make_identity

B, T, C = 4, 1024, 1024
NH = 16
HD = 64
MEM = 64
SCALE = 1.0 / float(np.sqrt(HD))
F32 = mybir.dt.float32
F32R = mybir.dt.float32r
EXP = mybir.ActivationFunctionType.Exp
SQRT = mybir.ActivationFunctionType.Sqrt
SQUARE = mybir.ActivationFunctionType.Square
GE = mybir.AluOpType.is_ge
AXX = mybir.AxisListType.X

# The walrus build in this environment rejects instructions carrying more
# than one sync-wait command (setupSyncWait in CoreV3GenImpl). Tile freely
# emits multi-wait instructions, so after scheduling we rewrite the BIR:
# each excess wait moves to a fresh NoOp on the same engine placed directly
# before the instruction. Engines execute their stream in order, so the
# accumulated waits gate the instruction exactly as before.
_MAX_WAITS = 1


def _split_waits(nc, max_waits=_MAX_WAITS):
    n_new = 0
    for fn in nc.m.functions:
        for bb in fn.blocks:
            out = []
            changed = False
            for inst in bb.instructions:
                si = inst.sync_info
                if si is not None:
                    waits = list(si.on_wait)
                    if len(waits) > max_waits:
                        changed = True
                        keep = len(waits) - max_waits
                        for i, w in enumerate(waits[:keep]):
                            nop = mybir.InstNoOp(
                                name=f"{inst.name}__w{i}", ins=[], outs=[]
                            )
                            nop.engine = inst.engine
                            nop.sync_info = bass_rust.SyncInfo(
                                on_wait=[w], on_update=[]
                            )
                            nc.register_instruction(nop)
                            out.append(nop)
                            n_new += 1
                        si.on_wait = waits[keep:]
                out.append(inst)
            if changed:
                bb.instructions = out
    return n_new


def _r(ap):
    return ap.bitcast(F32R)


def build_nc(phases=3):
    nc = bass.Bass("TRN2", target_bir_lowering=False, debug=False, num_devices=8)
    xT_d = nc.dram_tensor("xT", [C, B * T], F32R, kind="ExternalInput").ap()
    wa_d = nc.dram_tensor("wa", [C, 384], F32R, kind="ExternalInput").ap()
    wp_d = nc.dram_tensor("wp", [128, C], F32R, kind="ExternalInput").ap()
    att_d = nc.dram_tensor("att_c", [B, 2, T, T], F32, kind="ExternalOutput").ap()
    std_d = nc.dram_tensor("std_c", [2, T, T], F32, kind="ExternalOutput").ap()
    yp_d = nc.dram_tensor("yp_c", [B, T, C], F32, kind="ExternalOutput").ap()
    NEG = -700.0  # additive mask: exp((s + NEG) * SCALE) == 0 in fp32

    # xT viewed as [ki, ko, b, jblk, 128]: phase 1 streams column-block jblk
    # for ALL batches at once, so attention on block jblk can start as soon
    # as blocks <= jblk are done.
    xT_r = xT_d.rearrange(
        "(ko ki) (bb jb t) -> ki ko bb jb t", ki=128, bb=B, t=128
    )

    with tile.TileContext(nc) as tc:
        with (
            tc.tile_pool(name="singles", bufs=1) as singles,
            tc.tile_pool(name="p1", bufs=3) as p1,
            tc.tile_pool(name="p2", bufs=2) as p2,
            tc.tile_pool(name="p2e", bufs=4) as p2e,
            tc.tile_pool(name="p3", bufs=2) as p3,
            tc.tile_pool(name="ps", bufs=2, space="PSUM") as ps,
        ):
            wa_sb = singles.tile([128, 8, 384], F32R)
            nc.sync.dma_start(
                wa_sb[:], wa_d.rearrange("(ko ki) f -> ki ko f", ki=128)
            )
            wp_sb = singles.tile([128, C], F32R)
            nc.sync.dma_start(wp_sb[:], wp_d)
            # W_proj rows 64:128 replicated at partitions 0:64 so the h1 proj
            # matmul can run with both operands at partition base 0.
            wp2_sb = singles.tile([64, C], F32R)
            nc.sync.dma_start(wp2_sb[:], wp_d[64:128, :])
            q_sb = singles.tile([128, B * T], F32R)
            k_sb = singles.tile([128, B * T], F32R)
            v_sb = singles.tile([128, 32, 2, 65], F32R)
            nc.vector.memset(v_sb[:, :, :, 64:65].bitcast(F32), 1.0)
            ones_sb = singles.tile([128, 64], F32R)
            nc.vector.memset(ones_sb[:].bitcast(F32), 1.0)
            ident = singles.tile([128, 128], F32)
            make_identity(nc, ident[:])
            scratch = singles.tile([128, 512], F32)
            ident_r = singles.tile([128, 128], F32R)
            make_identity(nc, scratch[:, 0:128])
            nc.vector.tensor_copy(ident_r[:], scratch[:, 0:128])
            # Masks are ADDED to the psum score tiles via accumulating
            # identity-matmuls on the tensor engine.
            # maskw: [128, 512] with the causal diagonal mask (0 where j<=i,
            # NEG above) in the LAST 128 columns, zeros elsewhere, so the
            # mask-matmul covers the full range of the final score matmul.
            maskw = singles.tile([128, 512], F32R)
            mw = scratch[:]
            nc.vector.memset(mw, 0.0)
            nc.gpsimd.affine_select(
                mw[:, 384:512], mw[:, 384:512], [[-1, 128]], GE, NEG,
                base=0, channel_multiplier=1,
            )
            nc.vector.tensor_copy(maskw[:], mw)
            # additive band mask for the [j, i] band-score tile layout
            # segments: [0:128]=m1 (keep p - fi - 64 >= 0),
            # [128:384]=m0 and [384:512]=p1 (keep 0 <= i-j <= 64)
            band_madd = singles.tile([128, 512], F32R)
            bm = scratch[:]
            nc.vector.memset(bm, 0.0)
            nc.gpsimd.affine_select(
                bm[:, 0:128], bm[:, 0:128], [[-1, 128]], GE,
                NEG, base=-64, channel_multiplier=1,
            )
            nc.gpsimd.affine_select(
                bm[:, 128:384], bm[:, 128:384], [[1, 256]], GE,
                NEG, base=0, channel_multiplier=-1,
            )
            nc.gpsimd.affine_select(
                bm[:, 128:384], bm[:, 128:384], [[-1, 256]], GE,
                NEG, base=64, channel_multiplier=1,
            )
            nc.gpsimd.affine_select(
                bm[:, 384:512], bm[:, 384:512], [[1, 128]], GE,
                NEG, base=0, channel_multiplier=-1,
            )
            nc.gpsimd.affine_select(
                bm[:, 384:512], bm[:, 384:512], [[-1, 128]], GE,
                NEG, base=64, channel_multiplier=1,
            )
            nc.vector.tensor_copy(band_madd[:], bm)

            def phase1_block(jb):
                """qkv for column block jb (128 tokens of each batch)."""
                xt = p1.tile([128, 4, 4, 128], F32R, tag="xt", bufs=2)
                xt2 = p1.tile([128, 4, 4, 128], F32R, tag="xt2", bufs=2)
                for ko in range(4):
                    nc.sync.dma_start(
                        xt[:, ko, :, :], xT_r[:, ko, :, jb, :]
                    )
                    nc.sync.dma_start(
                        xt2[:, ko, :, :], xT_r[:, ko + 4, :, jb, :]
                    )
                qk = ps.tile([128, 1024], F32, tag="qk", bufs=1)
                def xsl(ko):
                    return xt[:, ko, :, :] if ko < 4 else xt2[:, ko - 4, :, :]
                for ko in range(8):
                    nc.tensor.matmul(
                        qk[:, 0:512], (wa_sb[:, ko, 0:128]), (xsl(ko)),
                        start=(ko == 0), stop=(ko == 7),
                    )
                for ko in range(8):
                    nc.tensor.matmul(
                        qk[:, 512:1024], (wa_sb[:, ko, 128:256]), (xsl(ko)),
                        start=(ko == 0), stop=(ko == 7),
                    )
                psv = ps.tile([128, 512], F32, tag="v", bufs=1)
                for ko in range(8):
                    nc.tensor.matmul(
                        psv[:], (wa_sb[:, ko, 256:384]), (xsl(ko)),
                        start=(ko == 0), stop=(ko == 7),
                    )
                # q/k columns for batch b live at b*T + jb*128; one strided
                # copy covers all four batches
                qv = q_sb[:].rearrange("p (bb jb t) -> p bb jb t", bb=B, t=128)
                kv = k_sb[:].rearrange("p (bb jb t) -> p bb jb t", bb=B, t=128)
                nc.vector.tensor_copy(
                    qv[:, :, jb, :], qk[:, 0:512].rearrange(
                        "p (bb t) -> p bb t", bb=B)
                )
                nc.scalar.copy(
                    kv[:, :, jb, :], qk[:, 512:1024].rearrange(
                        "p (bb t) -> p bb t", bb=B)
                )
                vT = p1.tile([128, 512], F32, tag="vT", bufs=2)
                nc.scalar.copy(vT[:], psv[:])
                for b in range(B):
                    pst = ps.tile([128, 128], F32, tag="v", bufs=1)
                    nc.tensor.transpose(pst[:], vT[:, ts(b, 128)], ident[:])
                    tb = b * 8 + jb
                    nc.vector.tensor_copy(v_sb[:, tb, 0, 0:64], pst[:, 0:64])
                    nc.vector.tensor_copy(
                        v_sb[:, tb, 1, 0:64], pst[:, 64:128]
                    )

            def phase2_block(iblk):
                i0 = iblk * 128
                jw = i0 + 128
                w_last = jw - (jw - 1) // 512 * 512
                jt_last = jw - w_last
                for h in range(2):
                    hs = slice(h * 64, (h + 1) * 64)
                    att4 = p2.tile([128, 4, 1024], F32, tag="att4")
                    Es = []
                    rs = p2e.tile([128, 4], F32, tag="rs")
                    for b in range(B):
                        g0 = b * T
                        q_lhsT = q_sb[hs, g0 + i0 : g0 + i0 + 128]
                        E = p2e.tile([128, 1024], F32, tag="E")
                        rs2 = p2e.tile([128, 2], F32, tag="rs2")
                        nchunk = (jw + 511) // 512
                        for ci, jt in enumerate(range(0, jw, 512)):
                            w = min(512, jw - jt)
                            psc = ps.tile([128, 512], F32, tag="sc", bufs=1)
                            nc.tensor.matmul(
                                psc[:, 0:w],
                                (q_lhsT),
                                (k_sb[hs, g0 + jt : g0 + jt + w]),
                                start=True, stop=(jt != jt_last),
                            )
                            if jt == jt_last:
                                nc.tensor.matmul(
                                    psc[:, 0:w], (ident_r[:]),
                                    (maskw[:, 512 - w_last : 512]),
                                    start=False, stop=True,
                                )
                            acc = (rs[:, b : b + 1] if nchunk == 1
                                   else rs2[:, ci : ci + 1])
                            nc.scalar.activation(
                                E[:, jt : jt + w], psc[:, 0:w], EXP,
                                scale=SCALE, accum_out=acc,
                            )
                        if nchunk > 1:
                            nc.vector.tensor_add(
                                rs[:, b : b + 1], rs2[:, 0:1], rs2[:, 1:2]
                            )
                        Es.append(E)
                    rinv = p2e.tile([128, 4], F32, tag="rinv")
                    nc.vector.reciprocal(rinv[:], rs[:])
                    for b in range(B):
                        eng = nc.gpsimd if b == 3 else nc.vector
                        eng.tensor_scalar_mul(
                            att4[:, b, :jw], Es[b][:, :jw],
                            rinv[:, b : b + 1],
                        )
                    deng = nc.sync if (iblk + h) % 2 == 0 else nc.scalar
                    deng.dma_start(
                        att_d[:, h, i0 : i0 + 128, 0:jw].rearrange(
                            "b p j -> p b j"
                        ),
                        att4[:, :, :jw],
                    )
                    # batch std, one-pass: V = S2 - S1*S1/4, std = sqrt(V/3)
                    a = att4[:, :, :jw]
                    sq = p2.tile([128, 4, 1024], F32, tag="sq", bufs=1)
                    nc.scalar.activation(sq[:, 0:4, :jw], a[:, 0:4], SQUARE)
                    tA = p2.tile([128, 1024], F32, tag="tA", bufs=1)
                    tB = p2.tile([128, 1024], F32, tag="tB", bufs=1)
                    tS = p2.tile([128, 1024], F32, tag="tS", bufs=1)
                    nc.gpsimd.tensor_add(tA[:, :jw], a[:, 0], a[:, 1])
                    nc.vector.tensor_add(tB[:, :jw], a[:, 2], a[:, 3])
                    nc.vector.tensor_add(tA[:, :jw], tA[:, :jw], tB[:, :jw])
                    nc.gpsimd.tensor_add(
                        tB[:, :jw], sq[:, 0, :jw], sq[:, 1, :jw]
                    )
                    nc.vector.tensor_add(
                        tS[:, :jw], sq[:, 2, :jw], sq[:, 3, :jw]
                    )
                    nc.vector.tensor_add(tS[:, :jw], tS[:, :jw], tB[:, :jw])
                    nc.vector.tensor_mul(tB[:, :jw], tA[:, :jw], tA[:, :jw])
                    nc.vector.scalar_tensor_tensor(
                        tA[:, :jw], tB[:, :jw], -0.25, tS[:, :jw],
                        op0=mybir.AluOpType.mult,
                        op1=mybir.AluOpType.add,
                    )
                    nc.vector.tensor_scalar_max(tA[:, :jw], tA[:, :jw], 0.0)
                    nc.scalar.activation(
                        tB[:, :jw], tA[:, :jw], SQRT, scale=1.0 / 3.0
                    )
                    deng2 = nc.scalar if (iblk + h) % 2 == 0 else nc.sync
                    deng2.dma_start(
                        std_d[h, i0 : i0 + 128, 0:jw], tB[:, :jw]
                    )

            def phase3_block(b, ipair):
                g0 = b * T
                i0 = ipair * 256
                yb = {}
                for h in range(2):
                    hs = slice(h * 64, (h + 1) * 64)
                    psb = ps.tile([128, 512], F32, tag="band", bufs=2)
                    # segments: [0:128]=m1, [128:384]=m0, [384:512]=p1
                    if ipair > 0:
                        nc.tensor.matmul(
                            psb[:, 0:128],
                            (k_sb[hs, g0 + i0 - 128 : g0 + i0]),
                            (q_sb[hs, g0 + i0 : g0 + i0 + 128]),
                            start=True, stop=False,
                        )
                        nc.tensor.matmul(
                            psb[:, 0:128], (ident_r[:]),
                            (band_madd[:, 0:128]),
                            start=False, stop=True,
                        )
                    nc.tensor.matmul(
                        psb[:, 128:384],
                        (k_sb[hs, g0 + i0 : g0 + i0 + 128]),
                        (q_sb[hs, g0 + i0 : g0 + i0 + 256]),
                        start=True, stop=False,
                    )
                    nc.tensor.matmul(
                        psb[:, 128:384], (ident_r[:]),
                        (band_madd[:, 128:384]),
                        start=False, stop=True,
                    )
                    nc.tensor.matmul(
                        psb[:, 384:512],
                        (k_sb[hs, g0 + i0 + 128 : g0 + i0 + 256]),
                        (q_sb[hs, g0 + i0 + 128 : g0 + i0 + 256]),
                        start=True, stop=False,
                    )
                    nc.tensor.matmul(
                        psb[:, 384:512], (ident_r[:]),
                        (band_madd[:, 384:512]),
                        start=False, stop=True,
                    )
                    lo = 0 if ipair > 0 else 128
                    ET = p3.tile([128, 512], F32R, tag="ET")
                    nc.scalar.activation(
                        ET[:, lo:512], psb[:, lo:512], EXP, scale=SCALE
                    )
                    # AV (+ bandsum via the ones column), [65, 256]
                    jt0 = b * 8 + ipair * 2
                    pav = ps.tile([128, 256], F32, tag="av", bufs=1)
                    nc.tensor.matmul(
                        pav[0:65, 0:256], (v_sb[:, jt0, h, :]),
                        (ET[:, 128:384]), start=True, stop=False,
                    )
                    if ipair > 0:
                        nc.tensor.matmul(
                            pav[0:65, 0:128], (v_sb[:, jt0 - 1, h, :]),
                            (ET[:, 0:128]), start=False, stop=False,
                        )
                    nc.tensor.matmul(
                        pav[0:65, 128:256], (v_sb[:, jt0 + 1, h, :]),
                        (ET[:, 384:512]), start=False, stop=True,
                    )
                    rb = p3.tile([128, 256], F32R, tag="rb")
                    with nc.allow_low_precision(
                        reason="f32r rounding for PE broadcast"
                    ):
                        nc.vector.reciprocal(rb[64:65, :], pav[64:65, :])
                    # broadcast rinv row to 64 partitions via a K=1 matmul:
                    # ones[1,64].T @ rinv[1,256] -> [64,256]
                    bc_ps = ps.tile([128, 256], F32, tag="band", bufs=2)
                    nc.tensor.matmul(
                        bc_ps[0:64, :], (ones_sb[64:65, :]),
                        (rb[64:65, :]), start=True, stop=True,
                        tile_position=(64, 0),
                    )
                    ybh = p3.tile([64, 256], F32R, tag="ybh")
                    nc.vector.tensor_copy(ybh[:], pav[0:64, :])
                    nc.vector.tensor_mul(ybh[:], ybh[:], bc_ps[0:64, :])
                    yb[h] = ybh
                for ic in range(2):
                    iblk = ipair * 2 + ic
                    yo = p3.tile([128, 1024], F32, tag="yo")
                    for ch in range(2):
                        psp = ps.tile([128, 512], F32, tag="proj", bufs=1)
                        nc.tensor.matmul(
                            psp[:],
                            (yb[0][:, ts(ic, 128)]),
                            (wp_sb[0:64, ts(ch, 512)]),
                            start=True, stop=False,
                        )
                        nc.tensor.matmul(
                            psp[:],
                            (yb[1][:, ts(ic, 128)]),
                            (wp2_sb[:, ts(ch, 512)]),
                            start=False, stop=True,
                        )
                        nc.scalar.copy(yo[:, ts(ch, 512)], psp[:])
                    nc.sync.dma_start(
                        yp_d[b, iblk * 128 : (iblk + 1) * 128, :], yo[:]
                    )

            for jb in range(8):
                phase1_block(jb)
                if phases >= 2:
                    phase2_block(jb)
                if phases >= 3 and jb % 2 == 1:
                    for b in range(B):
                        phase3_block(b, jb // 2)
    _split_waits(nc)
    return nc


def _shard_inputs(x, W_attn, W_proj):
    x = np.asarray(x, dtype=np.float32)
    W_attn = np.asarray(W_attn, dtype=np.float32)
    W_proj = np.asarray(W_proj, dtype=np.float32)
    xT = np.ascontiguousarray(x.reshape(B * T, C).T)
    in_maps = []
    for c in range(8):
        h0, h1 = 2 * c, 2 * c + 1
        wa = np.ascontiguousarray(
            np.concatenate(
                [
                    W_attn[:, h0 * 64 : (h0 + 1) * 64],
                    W_attn[:, h1 * 64 : (h1 + 1) * 64],
                    W_attn[:, C + h0 * 64 : C + (h0 + 1) * 64],
                    W_attn[:, C + h1 * 64 : C + (h1 + 1) * 64],
                    W_attn[:, 2 * C + h0 * 64 : 2 * C + (h0 + 1) * 64],
                    W_attn[:, 2 * C + h1 * 64 : 2 * C + (h1 + 1) * 64],
                ],
                axis=1,
            )
        )
        wp = np.ascontiguousarray(W_proj[c * 128 : (c + 1) * 128, :])
        in_maps.append({"xT": xT, "wa": wa, "wp": wp})
    return in_maps


def kernel(x, W_attn, W_proj, _trace=False):
    nc = build_nc()
    in_maps = _shard_inputs(x, W_attn, W_proj)
    res = None
    for attempt in range(4):
        try:
            res = run_bass_kernel_spmd(
                nc, in_maps, core_ids=list(range(8)), trace=_trace
            )
            break
        except Exception:
            if attempt == 3:
                raise
            import time as _time
            _time.sleep(3.0)
    att = np.concatenate(
        [r["att_c"] for r in res.results], axis=1
    )  # [B, 16, T, T]
    att_std = np.concatenate([r["std_c"] for r in res.results], axis=0)
    y = res.results[0]["yp_c"].copy()
    for r in res.results[1:]:
        y += r["yp_c"]
    kernel.last_results = res
    return (y, att, att_std)
